# revision 37
# baseline (speedup 1.0000x reference)
"""Trainium2 Bass kernel for nn_CCNNCAModel (RFF + convexified chunk attention).

Contract: kernel(Z, W_rff, A) takes FULL inputs, returns the FULL output tuple
(predictions [N,3], AW [N,3], reg_loss scalar, alpha [256]) matching
reference.py. Pure data-parallel shard of Z rows over 8 NeuronCores; the only
cross-core communication is an AllReduce of the [512] per-column trig sums
feeding the chunk-attention scores.

Per core (M = N/8 rows):

  proj^T = W^T Z^T on PE as an exact fp16-split product (full fp16 rate):
      Z = Zh + Zl, W = Wh + Wl (fp16 head + fp16 residual)
      proj = Zh Wh (K=64) + [Zh;Zl]-stacked cross term (K=128)
      (dropped Zl*Wl term is ~2^-22 relative -> proj error ~2e-6 rad)
  range reduction (the ACT Sin table only covers ~[-3.3, 3.3]):
      k = rint(proj/2pi) stored as int8 (fp32->int convert rounds to nearest;
      cached in SBUF, phase 2 reuses it), m = proj - 2pi*k          [DVE]
      |m| = bitwise_and(m, 0x7fffffff) on the int32 view       [DVE 2x mode]
  sin(x) = Sin(m), cos(x) = Sin(-|m| + pi/2) on ACT. Phase 1 runs on 1024-row
      tiles and uses accum_out to get per-column sums for free.
  AllReduce [128x4] column sums -> scores -> softmax alpha -> fold alpha and
      the sqrt(1/R) Q scaling into A (tiny per-row scale of [512,3]).
  phase 2 (512-row tiles): recompute trig (k cached), preds^T [3,512] =
      sum_t A_t^T Q_t (fp32 PE), stage via SBUF->SBUF DMA into a
      [128, M/128, 3] tile, then Exp/softmax row-wise and write predictions
      and AW in natural [M,3] layout.
"""

import sys

sys.path.insert(0, "/opt/trn_rl_repo")

import numpy as np
import ml_dtypes

import concourse.bass as bass
import concourse.mybir as mybir
import concourse.tile as tile
from concourse import bacc
from concourse.masks import make_identity

N_CORES = 8
D_IN = 64
RFF_DIM = 256
MT = 512  # phase-2 row-tile
MT1 = 1024  # phase-1 row-tile

f32 = mybir.dt.float32
i32 = mybir.dt.int32
i8 = mybir.dt.int8
f16 = mybir.dt.float16
A_OP = mybir.AluOpType
ACT_F = mybir.ActivationFunctionType

TWO_PI = float(2.0 * np.pi)
INV_2PI = float(1.0 / (2.0 * np.pi))
HALF_PI = float(np.pi / 2.0)
MAGIC = float(1.5 * 2**23)
SIN_SCALE = float(1.0 - 2e-6)  # sim-only: keep |scale*m| strictly inside pi


def build_module(
    m: int,
    n_cores: int = N_CORES,
    reduce_mode: str = "k8",
    reps: int = 1,
    use_cc: bool = True,
):
    """Per-core SPMD Bass module. reduce_mode: "k8" (HW) or "magic" (sim).
    reps > 1 wraps the whole body in a hardware For loop (timing only)."""
    ntiles = m // MT
    assert m % MT1 == 0 and m % 128 == 0
    nt1 = m // MT1
    mm = m // 128

    nc = bacc.Bacc(
        "TRN2", target_bir_lowering=False, debug=False, num_devices=n_cores
    )
    z2_d = nc.dram_tensor("z2", [128, m], f16, kind="ExternalInput")
    wh_d = nc.dram_tensor("wh", [D_IN, RFF_DIM], f16, kind="ExternalInput")
    wx_d = nc.dram_tensor("wx", [128, RFF_DIM], f16, kind="ExternalInput")
    a4_d = nc.dram_tensor("a4", [128, 12], f32, kind="ExternalInput")
    c01_d = nc.dram_tensor("c01", [4, 2], f32, kind="ExternalInput")
    preds_d = nc.dram_tensor("preds", [m, 3], f32, kind="ExternalOutput")
    aw_d = nc.dram_tensor("aw", [m, 3], f32, kind="ExternalOutput")
    alpha_d = nc.dram_tensor("alpha", [1, RFF_DIM], f32, kind="ExternalOutput")

    with tile.TileContext(nc) as tc:
        with (
            tc.tile_pool(name="singles", bufs=1) as singles,
            tc.tile_pool(name="glue", bufs=1) as glue,
            tc.tile_pool(name="dramp", bufs=1, space="DRAM") as dramp,
        ):
            # ---- persistent loads ----
            wh = singles.tile([D_IN, RFF_DIM], f16)
            nc.sync.dma_start(out=wh, in_=wh_d.ap())
            wx = singles.tile([128, RFF_DIM], f16)
            nc.sync.dma_start(out=wx, in_=wx_d.ap())
            z2 = singles.tile([128, m], f16)
            zch = min(8192, m)
            for j in range(0, m, zch):
                nc.sync.dma_start(
                    out=z2[:, j : j + zch], in_=z2_d.ap()[:, j : j + zch]
                )
            a4 = singles.tile([128, 12], f32)
            nc.sync.dma_start(out=a4, in_=a4_d.ap())
            c01 = singles.tile([4, 2], f32)
            nc.sync.dma_start(out=c01, in_=c01_d.ap())
            halfpi = singles.tile([128, 1], f32)
            nc.vector.memset(halfpi, HALF_PI)
            one1 = singles.tile([1, 1], f32)
            nc.vector.memset(one1, 1.0)
            ident = singles.tile([128, 128], f32)
            make_identity(nc, ident[:])
            accs = singles.tile([128, 4 * nt1], f32)
            if reduce_mode == "k8":
                # layout: col = i1*2*MT1 + h*MT1 + q  (q = row within tile i1)
                k8_all = singles.tile([128, 2 * m], i8)

            def emit_proj(pool, rows0, width):
                """proj^T rows [rows0, rows0+width) -> [128, 2*width] PSUM
                tile laid out [h, q] (feature half major)."""
                pj = pool.tile([128, 2 * width], f32, tag="pj")
                for h in range(2):
                    wsl = bass.ds(h * 128, 128)
                    for s in range(width // MT):
                        sl = bass.ds(rows0 + s * MT, MT)
                        out_sl = pj[:, h * width + s * MT : h * width + (s + 1) * MT]
                        nc.tensor.matmul(
                            out_sl, lhsT=wh[:, wsl], rhs=z2[0:D_IN, sl],
                            start=True, stop=False,
                        )
                        nc.tensor.matmul(
                            out_sl, lhsT=wx[:, wsl], rhs=z2[:, sl],
                            start=False, stop=True,
                        )
                return pj

            def emit_reduce(pool, pj, width, k_in, k_out):
                """Range reduction; returns (mr, am, sin_scale)."""
                mr = pool.tile([128, 2 * width], f32, tag="mr")
                if reduce_mode == "k8":
                    if k_out is not None:
                        nc.vector.tensor_scalar_mul(k_out, pj, INV_2PI)
                        k_in = k_out
                    nc.vector.scalar_tensor_tensor(
                        out=mr, in0=k_in, scalar=-TWO_PI, in1=pj,
                        op0=A_OP.mult, op1=A_OP.add,
                    )
                    sscale = 1.0
                else:
                    t = pool.tile([128, 2 * width], f32, tag="tmagic")
                    nc.vector.tensor_scalar(
                        out=t, in0=pj, scalar1=INV_2PI, scalar2=MAGIC,
                        op0=A_OP.mult, op1=A_OP.add,
                    )
                    k2pi = pool.tile([128, 2 * width], f32, tag="k2pi")
                    nc.gpsimd.tensor_scalar(
                        out=k2pi, in0=t, scalar1=MAGIC, scalar2=TWO_PI,
                        op0=A_OP.subtract, op1=A_OP.mult,
                    )
                    nc.vector.tensor_sub(mr, pj, k2pi)
                    sscale = SIN_SCALE
                am = pool.tile([128, 2 * width], f32, tag="am")
                nc.vector.tensor_scalar(
                    out=am.bitcast(i32), in0=mr.bitcast(i32),
                    scalar1=0x7FFFFFFF, scalar2=None, op0=A_OP.bitwise_and,
                )
                return mr, am, sscale

            from contextlib import ExitStack, nullcontext

            rep_ctx = ExitStack()
            if reps > 1:
                rep_ctx.enter_context(tc.For_i(0, reps, 1))
            # ---- phase 1: column sums of raw cos/sin (1024-row tiles) ----
            with (
                tc.tile_pool(name="mp1", bufs=2) as mp1,
                tc.tile_pool(name="qp1", bufs=2) as qp1,
                tc.tile_pool(name="pp1", bufs=2, space="PSUM") as pp1,
            ):
                for i1 in range(nt1):
                    pj = emit_proj(pp1, i1 * MT1, MT1)
                    k_out = (
                        k8_all[:, i1 * 2 * MT1 : (i1 + 1) * 2 * MT1]
                        if reduce_mode == "k8"
                        else None
                    )
                    mr, am, sscale = emit_reduce(mp1, pj, MT1, None, k_out)
                    cosp = qp1.tile([128, MT1], f32, tag="cosp")
                    sinp = qp1.tile([128, MT1], f32, tag="sinp")
                    for h in range(2):
                        hsl = bass.ds(h * MT1, MT1)
                        nc.scalar.activation(
                            out=cosp, in_=am[:, hsl], func=ACT_F.Sin,
                            bias=halfpi[:], scale=-1.0,
                            accum_out=accs[:, h * nt1 + i1 : h * nt1 + i1 + 1],
                        )
                        nc.scalar.activation(
                            out=sinp, in_=mr[:, hsl], func=ACT_F.Sin,
                            scale=sscale,
                            accum_out=accs[
                                :, (2 + h) * nt1 + i1 : (2 + h) * nt1 + i1 + 1
                            ],
                        )

            colsum = glue.tile([128, 4], f32)
            for c in range(4):
                nc.vector.reduce_sum(
                    colsum[:, c : c + 1],
                    accs[:, c * nt1 : (c + 1) * nt1],
                    axis=mybir.AxisListType.X,
                )

            # ---- AllReduce over cores ----
            if n_cores > 1 and use_cc:
                cin = dramp.tile([128, 4], f32)
                cout = dramp.tile([128, 4], f32)
                nc.sync.dma_start(out=cin, in_=colsum)
                nc.gpsimd.collective_compute(
                    "AllReduce",
                    A_OP.add,
                    replica_groups=[list(range(n_cores))],
                    ins=[cin.opt()],
                    outs=[cout.opt()],
                )
                colg = glue.tile([128, 4], f32)
                nc.sync.dma_start(out=colg, in_=cout)
            else:
                colg = colsum

            with (
                tc.tile_pool(name="mp2", bufs=3) as mp2,
                tc.tile_pool(name="qp2", bufs=3) as qp2,
                tc.tile_pool(name="pp2", bufs=2, space="PSUM") as pp2,
                tc.tile_pool(name="ppd", bufs=2, space="PSUM") as ppd,
                tc.tile_pool(name="pglue", bufs=1, space="PSUM") as pglue,
            ):
                # ---- scores -> alpha ----
                xps = pglue.tile([4, 128], f32)
                nc.tensor.transpose(xps, colg, ident[:])
                xsb = glue.tile([4, 128], f32)
                nc.vector.tensor_copy(xsb, xps)
                x2 = xsb.rearrange("c (f two) -> c f two", two=2)
                t0 = glue.tile([4, 64], f32)
                nc.vector.tensor_scalar_mul(t0, x2[:, :, 0], c01[:, 0:1])
                sc = glue.tile([4, 64], f32)
                nc.vector.scalar_tensor_tensor(
                    out=sc, in0=x2[:, :, 1], scalar=c01[:, 1:2], in1=t0,
                    op0=A_OP.mult, op1=A_OP.add,
                )
                aflat = glue.tile([1, 256], f32)
                nc.sync.dma_start(
                    out=aflat.rearrange("o (c f) -> o c f", c=4), in_=sc
                )
                mx = glue.tile([1, 1], f32)
                nc.vector.reduce_max(mx, aflat, axis=mybir.AxisListType.X)
                nmx = glue.tile([1, 1], f32)
                nc.vector.tensor_scalar_mul(nmx, mx, -1.0)
                ev = glue.tile([1, 256], f32)
                se = glue.tile([1, 1], f32)
                nc.scalar.activation(
                    out=ev, in_=aflat, func=ACT_F.Exp, bias=nmx[:], scale=1.0,
                    accum_out=se,
                )
                rs = glue.tile([1, 1], f32)
                nc.vector.reciprocal(rs, se)
                alph = glue.tile([1, 256], f32)
                nc.vector.tensor_scalar_mul(alph, ev, rs[:, 0:1])
                nc.sync.dma_start(out=alpha_d.ap(), in_=alph)

                # ---- a_s[p, t, c] = alpha[(128t+p)//2] * a4[p, t, c] ----
                aexp = glue.tile([1, 512], f32)
                aet = aexp.rearrange("o (f two) -> o two f", two=2)
                nc.vector.tensor_copy(aet[:, 0, :], alph)
                nc.vector.tensor_copy(aet[:, 1, :], alph)
                sps = pglue.tile([128, 4], f32)
                for t in range(4):
                    nc.tensor.matmul(
                        sps[:, t : t + 1],
                        lhsT=aexp[:, t * 128 : (t + 1) * 128],
                        rhs=one1[:],
                        start=True,
                        stop=True,
                    )
                ssb = glue.tile([128, 4], f32)
                nc.vector.tensor_copy(ssb, sps)
                a_s = glue.tile([128, 12], f32)
                a4v = a4.rearrange("p (t c) -> p t c", t=4)
                asv = a_s.rearrange("p (t c) -> p t c", t=4)
                for t in range(4):
                    nc.vector.tensor_scalar_mul(
                        asv[:, t, :], a4v[:, t, :], ssb[:, t : t + 1]
                    )

                # ---- phase 2a: predictions^T per 512-row tile ----
                pb = dramp.tile([3, m], f32)
                if reduce_mode == "k8":
                    k8v = k8_all.rearrange(
                        "p (i1 h q) -> p i1 h q", h=2, q=MT1
                    )
                for i in range(ntiles):
                    pj = emit_proj(pp2, i * MT, MT)
                    if reduce_mode == "k8":
                        i1, s = i * MT // MT1, (i * MT % MT1) // MT
                        k_in = k8v[:, i1, :, s * MT : (s + 1) * MT]
                        pj_v = pj.rearrange("p (h q) -> p h q", h=2)
                        mr = mp2.tile([128, 2 * MT], f32, tag="mr")
                        nc.vector.scalar_tensor_tensor(
                            out=mr.rearrange("p (h q) -> p h q", h=2),
                            in0=k_in, scalar=-TWO_PI, in1=pj_v,
                            op0=A_OP.mult, op1=A_OP.add,
                        )
                        am = mp2.tile([128, 2 * MT], f32, tag="am")
                        nc.vector.tensor_scalar(
                            out=am.bitcast(i32), in0=mr.bitcast(i32),
                            scalar1=0x7FFFFFFF, scalar2=None,
                            op0=A_OP.bitwise_and,
                        )
                        sscale = 1.0
                    else:
                        mr, am, sscale = emit_reduce(mp2, pj, MT, None, None)
                    cosp = qp2.tile([128, 2 * MT], f32, tag="cosp")
                    sinp = qp2.tile([128, 2 * MT], f32, tag="sinp")
                    nc.scalar.activation(
                        out=cosp, in_=am, func=ACT_F.Sin,
                        bias=halfpi[:], scale=-1.0,
                    )
                    nc.scalar.activation(
                        out=sinp, in_=mr, func=ACT_F.Sin, scale=sscale
                    )
                    q_chunks = [
                        cosp[:, 0:MT], cosp[:, MT : 2 * MT],
                        sinp[:, 0:MT], sinp[:, MT : 2 * MT],
                    ]
                    pd = ppd.tile([3, MT], f32, tag="pd")
                    for t in range(4):
                        nc.tensor.matmul(
                            pd, lhsT=asv[:, t, :], rhs=q_chunks[t],
                            start=(t == 0), stop=(t == 3),
                        )
                    pt = qp2.tile([3, MT], f32, tag="pt")
                    nc.vector.tensor_copy(pt, pd)
                    # bounce preds^T through DRAM with clean contiguous APs
                    nc.sync.dma_start(
                        out=pb[:, bass.ds(i * MT, MT)], in_=pt
                    )

                # ---- phase 2b: Exp/softmax + natural-layout outputs ----
                # load in c-major (1KB runs), then one strided DVE copy to
                # the q-major n = a*mm + q layout used for contiguous output
                pbs_c = glue.tile([128, 3, mm], f32)
                nc.sync.dma_start(
                    out=pbs_c, in_=pb.rearrange("c (a q) -> a c q", a=128)
                )
                pbs = glue.tile([128, mm, 3], f32)
                nc.vector.tensor_copy(
                    pbs.rearrange("p q c -> p c q"), pbs_c
                )
                nc.sync.dma_start(
                    out=preds_d.ap().rearrange("(a q) c -> a (q c)", a=128),
                    in_=pbs.rearrange("p q c -> p (q c)"),
                )
                e3 = glue.tile([128, mm, 3], f32)
                nc.scalar.activation(
                    out=e3, in_=pbs, func=ACT_F.Exp, scale=1.0 / 16.0
                )
                ssum = glue.tile([128, mm], f32)
                nc.vector.reduce_sum(ssum, e3, axis=mybir.AxisListType.X)
                rr = glue.tile([128, mm], f32)
                nc.vector.reciprocal(rr, ssum)
                aw3 = glue.tile([128, mm, 3], f32)
                for c in range(3):
                    nc.vector.tensor_mul(aw3[:, :, c], e3[:, :, c], rr)
                nc.sync.dma_start(
                    out=aw_d.ap().rearrange("(a q) c -> a (q c)", a=128),
                    in_=aw3.rearrange("p q c -> p (q c)"),
                )
            rep_ctx.close()

    nc.compile()
    return nc


def make_in_maps(Z, W_rff, A, n_cores: int = N_CORES):
    """Host-side prep: fp16-split + transpose Z, fold constants."""
    Z = np.ascontiguousarray(np.asarray(Z, dtype=np.float32))
    W_rff = np.ascontiguousarray(np.asarray(W_rff, dtype=np.float32))
    A = np.ascontiguousarray(np.asarray(A, dtype=np.float32))
    n_total = Z.shape[0]
    m = n_total // n_cores

    hp = np.float16
    Zh = Z.astype(hp)
    Zl = (Z - Zh.astype(np.float32)).astype(hp)
    z2 = np.ascontiguousarray(np.concatenate([Zh.T, Zl.T], axis=0))
    Wh = W_rff.astype(hp)
    Wl = (W_rff - Wh.astype(np.float32)).astype(hp)
    wx = np.ascontiguousarray(np.concatenate([Wl, Wh], axis=0))

    a_div = (A / 16.0).astype(np.float32)
    a4 = np.ascontiguousarray(
        a_div.reshape(4, 128, 3).transpose(1, 0, 2).reshape(128, 12)
    )
    a_mean = A.mean(axis=1)
    denom = 16.0 * float(n_total) * (0.1 * np.sqrt(RFF_DIM))
    c01 = np.tile(
        np.array([[a_mean[0] / denom, a_mean[1] / denom]], dtype=np.float32),
        (4, 1),
    )
    in_maps = []
    for core in range(n_cores):
        in_maps.append(
            {
                "z2": np.ascontiguousarray(z2[:, core * m : (core + 1) * m]),
                "wh": np.ascontiguousarray(Wh),
                "wx": wx,
                "a4": a4,
                "c01": c01,
            }
        )
    return in_maps, m


_CACHE = {}


def _get_runner(
    m: int, n_cores: int = N_CORES, reduce_mode: str = "k8", reps: int = 1
):
    """Compile once; return (run_fn, put_fn, jitted). Mirrors the multi-core
    branch of bass2jax.run_bass_via_pjrt but caches the jitted callable."""
    key = (m, n_cores, reduce_mode, reps)
    if key in _CACHE:
        return _CACHE[key]

    import jax
    from jax.sharding import Mesh, PartitionSpec, NamedSharding
    from jax.experimental.shard_map import shard_map
    import concourse.mybir as mybir_
    from concourse import bass2jax

    nc = build_module(m, n_cores, reduce_mode, reps=reps, use_cc=(reps == 1))
    bass2jax.install_neuronx_cc_hook()

    partition_name = (
        nc.partition_id_tensor.name if nc.partition_id_tensor else None
    )
    in_names, out_names, out_avals, zero_outs = [], [], [], []
    for alloc in nc.m.functions[0].allocations:
        if not isinstance(alloc, mybir_.MemoryLocationSet):
            continue
        name = alloc.memorylocations[0].name
        if alloc.kind == "ExternalInput":
            if name != partition_name:
                in_names.append(name)
        elif alloc.kind == "ExternalOutput":
            shape = tuple(alloc.tensor_shape)
            dtype = mybir_.dt.np(alloc.dtype)
            out_names.append(name)
            out_avals.append(jax.core.ShapedArray(shape, dtype))
            zero_outs.append(np.zeros(shape, dtype))
    n_params = len(in_names)
    all_in_names = list(in_names) + list(out_names)
    if partition_name is not None:
        all_in_names.append(partition_name)

    def _body(*args):
        operands = list(args)
        if partition_name is not None:
            operands.append(bass2jax.partition_id_tensor())
        outs = bass2jax._bass_exec_p.bind(
            *operands,
            out_avals=tuple(out_avals),
            in_names=tuple(all_in_names),
            out_names=tuple(out_names),
            lowering_input_output_aliases=(),
            sim_require_finite=True,
            sim_require_nnan=True,
            nc=nc,
        )
        return tuple(outs)

    devices = jax.devices()[:n_cores]
    mesh = Mesh(np.asarray(devices), ("core",))
    in_specs = (PartitionSpec("core"),) * (n_params + len(out_names))
    out_specs = (PartitionSpec("core"),) * len(out_names)
    sharded = jax.jit(
        shard_map(_body, mesh=mesh, in_specs=in_specs, out_specs=out_specs,
                  check_rep=False),
        keep_unused=True,
    )
    sharding = NamedSharding(mesh, PartitionSpec("core"))

    def put_fn(in_maps):
        import jax as _jax

        concat_in = [
            np.concatenate([in_maps[c][nm] for c in range(n_cores)], axis=0)
            for nm in in_names
        ]
        concat_zeros = [
            np.zeros((n_cores * z.shape[0], *z.shape[1:]), z.dtype)
            for z in zero_outs
        ]
        return [
            _jax.device_put(a, sharding) for a in concat_in + concat_zeros
        ]

    def run_fn(device_args):
        out_arrs = sharded(*device_args)
        out_arrs = [np.asarray(o) for o in out_arrs]
        return [
            {
                nm: out_arrs[i].reshape(n_cores, *out_avals[i].shape)[c]
                for i, nm in enumerate(out_names)
            }
            for c in range(n_cores)
        ]

    _CACHE[key] = (run_fn, put_fn, sharded)
    return _CACHE[key]


def kernel(Z, W_rff, A):
    Z = np.asarray(Z, dtype=np.float32)
    A_np = np.asarray(A, dtype=np.float32)
    in_maps, m = make_in_maps(Z, W_rff, A_np, N_CORES)
    run_fn, put_fn, _ = _get_runner(m, N_CORES)
    results = run_fn(put_fn(in_maps))
    preds = np.concatenate([r["preds"] for r in results], axis=0)
    aw = np.concatenate([r["aw"] for r in results], axis=0)
    alpha = results[0]["alpha"].reshape(-1)
    reg_loss = np.float32(
        0.01 * np.linalg.svd(A_np, compute_uv=False).astype(np.float32).sum()
    )
    return preds, aw, reg_loss, alpha


# revision 38
# speedup vs baseline: 736.2713x; 736.2713x over previous
"""Trainium2 Bass kernel for nn_CCNNCAModel (RFF + convexified chunk attention).

Contract: kernel(Z, W_rff, A) takes FULL inputs, returns the FULL output tuple
(predictions [N,3], AW [N,3], reg_loss scalar, alpha [256]) matching
reference.py. Pure data-parallel shard of Z rows over 8 NeuronCores; the only
cross-core communication is an AllReduce of the [512] per-column trig sums
feeding the chunk-attention scores.

Per core (M = N/8 rows):

  proj^T = W^T Z^T on PE as an exact fp16-split product (full fp16 rate):
      Z = Zh + Zl, W = Wh + Wl (fp16 head + fp16 residual)
      proj = Zh Wh (K=64) + [Zh;Zl]-stacked cross term (K=128)
      (dropped Zl*Wl term is ~2^-22 relative -> proj error ~2e-6 rad)
  range reduction (the ACT Sin table only covers ~[-3.3, 3.3]):
      k = rint(proj/2pi) stored as int8 (fp32->int convert rounds to nearest;
      cached in SBUF, phase 2 reuses it), m = proj - 2pi*k          [DVE]
      |m| = bitwise_and(m, 0x7fffffff) on the int32 view       [DVE 2x mode]
  sin(x) = Sin(m), cos(x) = Sin(-|m| + pi/2) on ACT. Phase 1 runs on 1024-row
      tiles and uses accum_out to get per-column sums for free.
  AllReduce [128x4] column sums -> scores -> softmax alpha -> fold alpha and
      the sqrt(1/R) Q scaling into A (tiny per-row scale of [512,3]).
  phase 2 (512-row tiles): recompute trig (k cached), preds^T [3,512] =
      sum_t A_t^T Q_t (fp32 PE), stage via SBUF->SBUF DMA into a
      [128, M/128, 3] tile, then Exp/softmax row-wise and write predictions
      and AW in natural [M,3] layout.
"""

import sys

sys.path.insert(0, "/opt/trn_rl_repo")

import numpy as np
import ml_dtypes

import concourse.bass as bass
import concourse.mybir as mybir
import concourse.tile as tile
from concourse import bacc
from concourse.masks import make_identity

N_CORES = 8
D_IN = 64
RFF_DIM = 256
MT = 512  # phase-2 row-tile
MT1 = 1024  # phase-1 row-tile

f32 = mybir.dt.float32
i32 = mybir.dt.int32
i8 = mybir.dt.int8
f16 = mybir.dt.float16
A_OP = mybir.AluOpType
ACT_F = mybir.ActivationFunctionType

TWO_PI = float(2.0 * np.pi)
INV_2PI = float(1.0 / (2.0 * np.pi))
HALF_PI = float(np.pi / 2.0)
MAGIC = float(1.5 * 2**23)
SIN_SCALE = float(1.0 - 2e-6)  # sim-only: keep |scale*m| strictly inside pi


def build_module(
    m: int,
    n_cores: int = N_CORES,
    reduce_mode: str = "k8",
    reps: int = 1,
    use_cc: bool = True,
):
    """Per-core SPMD Bass module. reduce_mode: "k8" (HW) or "magic" (sim).
    reps > 1 wraps the whole body in a hardware For loop (timing only)."""
    ntiles = m // MT
    assert m % MT1 == 0 and m % 128 == 0
    nt1 = m // MT1
    mm = m // 128

    nc = bacc.Bacc(
        "TRN2", target_bir_lowering=False, debug=False, num_devices=n_cores
    )
    z2_d = nc.dram_tensor("z2", [128, m], f16, kind="ExternalInput")
    wh_d = nc.dram_tensor("wh", [D_IN, RFF_DIM], f16, kind="ExternalInput")
    wx_d = nc.dram_tensor("wx", [128, RFF_DIM], f16, kind="ExternalInput")
    a4_d = nc.dram_tensor("a4", [128, 12], f32, kind="ExternalInput")
    c01_d = nc.dram_tensor("c01", [4, 2], f32, kind="ExternalInput")
    preds_d = nc.dram_tensor("preds", [m, 3], f32, kind="ExternalOutput")
    aw_d = nc.dram_tensor("aw", [m, 3], f32, kind="ExternalOutput")
    alpha_d = nc.dram_tensor("alpha", [1, RFF_DIM], f32, kind="ExternalOutput")

    with tile.TileContext(nc) as tc:
        with (
            tc.tile_pool(name="singles", bufs=1) as singles,
            tc.tile_pool(name="glue", bufs=1) as glue,
            tc.tile_pool(name="dramp", bufs=1, space="DRAM") as dramp,
        ):
            # ---- persistent loads ----
            wh = singles.tile([D_IN, RFF_DIM], f16)
            nc.sync.dma_start(out=wh, in_=wh_d.ap())
            wx = singles.tile([128, RFF_DIM], f16)
            nc.sync.dma_start(out=wx, in_=wx_d.ap())
            z2 = singles.tile([128, m], f16)
            zch = min(8192, m)
            for j in range(0, m, zch):
                nc.sync.dma_start(
                    out=z2[:, j : j + zch], in_=z2_d.ap()[:, j : j + zch]
                )
            a4 = singles.tile([128, 12], f32)
            nc.sync.dma_start(out=a4, in_=a4_d.ap())
            c01 = singles.tile([4, 2], f32)
            nc.sync.dma_start(out=c01, in_=c01_d.ap())
            halfpi = singles.tile([128, 1], f32)
            nc.vector.memset(halfpi, HALF_PI)
            one1 = singles.tile([1, 1], f32)
            nc.vector.memset(one1, 1.0)
            ident = singles.tile([128, 128], f32)
            make_identity(nc, ident[:])
            accs = singles.tile([128, 4 * nt1], f32)
            if reduce_mode == "k8":
                # layout: col = i1*2*MT1 + h*MT1 + q  (q = row within tile i1)
                k8_all = singles.tile([128, 2 * m], i8)

            def emit_proj(pool, rows0, width):
                """proj^T rows [rows0, rows0+width) -> [128, 2*width] PSUM
                tile laid out [h, q] (feature half major)."""
                pj = pool.tile([128, 2 * width], f32, tag="pj")
                for h in range(2):
                    wsl = bass.ds(h * 128, 128)
                    for s in range(width // MT):
                        sl = bass.ds(rows0 + s * MT, MT)
                        out_sl = pj[:, h * width + s * MT : h * width + (s + 1) * MT]
                        nc.tensor.matmul(
                            out_sl, lhsT=wh[:, wsl], rhs=z2[0:D_IN, sl],
                            start=True, stop=False,
                        )
                        nc.tensor.matmul(
                            out_sl, lhsT=wx[:, wsl], rhs=z2[:, sl],
                            start=False, stop=True,
                        )
                return pj

            def emit_reduce(pool, pj, width, k_in, k_out):
                """Range reduction; returns (mr, am, sin_scale)."""
                mr = pool.tile([128, 2 * width], f32, tag="mr")
                if reduce_mode == "k8":
                    if k_out is not None:
                        nc.vector.tensor_scalar_mul(k_out, pj, INV_2PI)
                        k_in = k_out
                    nc.vector.scalar_tensor_tensor(
                        out=mr, in0=k_in, scalar=-TWO_PI, in1=pj,
                        op0=A_OP.mult, op1=A_OP.add,
                    )
                    sscale = 1.0
                else:
                    t = pool.tile([128, 2 * width], f32, tag="tmagic")
                    nc.vector.tensor_scalar(
                        out=t, in0=pj, scalar1=INV_2PI, scalar2=MAGIC,
                        op0=A_OP.mult, op1=A_OP.add,
                    )
                    k2pi = pool.tile([128, 2 * width], f32, tag="k2pi")
                    nc.gpsimd.tensor_scalar(
                        out=k2pi, in0=t, scalar1=MAGIC, scalar2=TWO_PI,
                        op0=A_OP.subtract, op1=A_OP.mult,
                    )
                    nc.vector.tensor_sub(mr, pj, k2pi)
                    sscale = SIN_SCALE
                am = pool.tile([128, 2 * width], f32, tag="am")
                nc.vector.tensor_scalar(
                    out=am.bitcast(i32), in0=mr.bitcast(i32),
                    scalar1=0x7FFFFFFF, scalar2=None, op0=A_OP.bitwise_and,
                )
                return mr, am, sscale

            from contextlib import ExitStack, nullcontext

            rep_ctx = ExitStack()
            if reps > 1:
                rep_ctx.enter_context(tc.For_i(0, reps, 1))
            # ---- phase 1: column sums of raw cos/sin (1024-row tiles) ----
            with (
                tc.tile_pool(name="mp1", bufs=2) as mp1,
                tc.tile_pool(name="qp1", bufs=2) as qp1,
                tc.tile_pool(name="pp1", bufs=2, space="PSUM") as pp1,
            ):
                for i1 in range(nt1):
                    pj = emit_proj(pp1, i1 * MT1, MT1)
                    k_out = (
                        k8_all[:, i1 * 2 * MT1 : (i1 + 1) * 2 * MT1]
                        if reduce_mode == "k8"
                        else None
                    )
                    mr, am, sscale = emit_reduce(mp1, pj, MT1, None, k_out)
                    cosp = qp1.tile([128, MT1], f32, tag="cosp")
                    sinp = qp1.tile([128, MT1], f32, tag="sinp")
                    for h in range(2):
                        hsl = bass.ds(h * MT1, MT1)
                        nc.scalar.activation(
                            out=cosp, in_=am[:, hsl], func=ACT_F.Sin,
                            bias=halfpi[:], scale=-1.0,
                            accum_out=accs[:, h * nt1 + i1 : h * nt1 + i1 + 1],
                        )
                        nc.scalar.activation(
                            out=sinp, in_=mr[:, hsl], func=ACT_F.Sin,
                            scale=sscale,
                            accum_out=accs[
                                :, (2 + h) * nt1 + i1 : (2 + h) * nt1 + i1 + 1
                            ],
                        )

            colsum = glue.tile([128, 4], f32)
            for c in range(4):
                nc.vector.reduce_sum(
                    colsum[:, c : c + 1],
                    accs[:, c * nt1 : (c + 1) * nt1],
                    axis=mybir.AxisListType.X,
                )

            # ---- AllReduce over cores ----
            if n_cores > 1 and use_cc:
                cin = dramp.tile([128, 4], f32)
                cout = dramp.tile([128, 4], f32)
                nc.sync.dma_start(out=cin, in_=colsum)
                nc.gpsimd.collective_compute(
                    "AllReduce",
                    A_OP.add,
                    replica_groups=[list(range(n_cores))],
                    ins=[cin.opt()],
                    outs=[cout.opt()],
                )
                colg = glue.tile([128, 4], f32)
                nc.sync.dma_start(out=colg, in_=cout)
            else:
                colg = colsum

            with (
                tc.tile_pool(name="mp2", bufs=3) as mp2,
                tc.tile_pool(name="qp2", bufs=3) as qp2,
                tc.tile_pool(name="pp2", bufs=2, space="PSUM") as pp2,
                tc.tile_pool(name="ppd", bufs=2, space="PSUM") as ppd,
                tc.tile_pool(name="pglue", bufs=1, space="PSUM") as pglue,
            ):
                # ---- scores -> alpha ----
                xps = pglue.tile([4, 128], f32)
                nc.tensor.transpose(xps, colg, ident[:])
                xsb = glue.tile([4, 128], f32)
                nc.vector.tensor_copy(xsb, xps)
                x2 = xsb.rearrange("c (f two) -> c f two", two=2)
                t0 = glue.tile([4, 64], f32)
                nc.vector.tensor_scalar_mul(t0, x2[:, :, 0], c01[:, 0:1])
                sc = glue.tile([4, 64], f32)
                nc.vector.scalar_tensor_tensor(
                    out=sc, in0=x2[:, :, 1], scalar=c01[:, 1:2], in1=t0,
                    op0=A_OP.mult, op1=A_OP.add,
                )
                aflat = glue.tile([1, 256], f32)
                nc.sync.dma_start(
                    out=aflat.rearrange("o (c f) -> o c f", c=4), in_=sc
                )
                mx = glue.tile([1, 1], f32)
                nc.vector.reduce_max(mx, aflat, axis=mybir.AxisListType.X)
                nmx = glue.tile([1, 1], f32)
                nc.vector.tensor_scalar_mul(nmx, mx, -1.0)
                ev = glue.tile([1, 256], f32)
                se = glue.tile([1, 1], f32)
                nc.scalar.activation(
                    out=ev, in_=aflat, func=ACT_F.Exp, bias=nmx[:], scale=1.0,
                    accum_out=se,
                )
                rs = glue.tile([1, 1], f32)
                nc.vector.reciprocal(rs, se)
                alph = glue.tile([1, 256], f32)
                nc.vector.tensor_scalar_mul(alph, ev, rs[:, 0:1])
                nc.sync.dma_start(out=alpha_d.ap(), in_=alph)

                # ---- a_s[p, t, c] = alpha[(128t+p)//2] * a4[p, t, c] ----
                aexp = glue.tile([1, 512], f32)
                aet = aexp.rearrange("o (f two) -> o two f", two=2)
                nc.vector.tensor_copy(aet[:, 0, :], alph)
                nc.vector.tensor_copy(aet[:, 1, :], alph)
                sps = pglue.tile([128, 4], f32)
                for t in range(4):
                    nc.tensor.matmul(
                        sps[:, t : t + 1],
                        lhsT=aexp[:, t * 128 : (t + 1) * 128],
                        rhs=one1[:],
                        start=True,
                        stop=True,
                    )
                ssb = glue.tile([128, 4], f32)
                nc.vector.tensor_copy(ssb, sps)
                a_s = glue.tile([128, 12], f32)
                a4v = a4.rearrange("p (t c) -> p t c", t=4)
                asv = a_s.rearrange("p (t c) -> p t c", t=4)
                for t in range(4):
                    nc.vector.tensor_scalar_mul(
                        asv[:, t, :], a4v[:, t, :], ssb[:, t : t + 1]
                    )

                # ---- phase 2a: predictions^T per 512-row tile ----
                pb = dramp.tile([3, m], f32)
                if reduce_mode == "k8":
                    k8v = k8_all.rearrange(
                        "p (i1 h q) -> p i1 h q", h=2, q=MT1
                    )
                for i in range(ntiles):
                    pj = emit_proj(pp2, i * MT, MT)
                    if reduce_mode == "k8":
                        i1, s = i * MT // MT1, (i * MT % MT1) // MT
                        k_in = k8v[:, i1, :, s * MT : (s + 1) * MT]
                        pj_v = pj.rearrange("p (h q) -> p h q", h=2)
                        mr = mp2.tile([128, 2 * MT], f32, tag="mr")
                        nc.vector.scalar_tensor_tensor(
                            out=mr.rearrange("p (h q) -> p h q", h=2),
                            in0=k_in, scalar=-TWO_PI, in1=pj_v,
                            op0=A_OP.mult, op1=A_OP.add,
                        )
                        am = mp2.tile([128, 2 * MT], f32, tag="am")
                        nc.vector.tensor_scalar(
                            out=am.bitcast(i32), in0=mr.bitcast(i32),
                            scalar1=0x7FFFFFFF, scalar2=None,
                            op0=A_OP.bitwise_and,
                        )
                        sscale = 1.0
                    else:
                        mr, am, sscale = emit_reduce(mp2, pj, MT, None, None)
                    cosp = qp2.tile([128, 2 * MT], f32, tag="cosp")
                    sinp = qp2.tile([128, 2 * MT], f32, tag="sinp")
                    nc.scalar.activation(
                        out=cosp, in_=am, func=ACT_F.Sin,
                        bias=halfpi[:], scale=-1.0,
                    )
                    nc.scalar.activation(
                        out=sinp, in_=mr, func=ACT_F.Sin, scale=sscale
                    )
                    q_chunks = [
                        cosp[:, 0:MT], cosp[:, MT : 2 * MT],
                        sinp[:, 0:MT], sinp[:, MT : 2 * MT],
                    ]
                    pd = ppd.tile([3, MT], f32, tag="pd")
                    for t in range(4):
                        nc.tensor.matmul(
                            pd, lhsT=asv[:, t, :], rhs=q_chunks[t],
                            start=(t == 0), stop=(t == 3),
                        )
                    pt = qp2.tile([3, MT], f32, tag="pt")
                    nc.vector.tensor_copy(pt, pd)
                    # bounce preds^T through DRAM with clean contiguous APs
                    nc.sync.dma_start(
                        out=pb[:, bass.ds(i * MT, MT)], in_=pt
                    )

                # ---- phase 2b: Exp/softmax + natural-layout outputs ----
                # load in c-major (1KB runs), then one strided DVE copy to
                # the q-major n = a*mm + q layout used for contiguous output
                pbs_c = glue.tile([128, 3, mm], f32)
                nc.sync.dma_start(
                    out=pbs_c, in_=pb.rearrange("c (a q) -> a c q", a=128)
                )
                pbs = glue.tile([128, mm, 3], f32)
                nc.vector.tensor_copy(
                    pbs.rearrange("p q c -> p c q"), pbs_c
                )
                nc.sync.dma_start(
                    out=preds_d.ap().rearrange("(a q) c -> a (q c)", a=128),
                    in_=pbs.rearrange("p q c -> p (q c)"),
                )
                e3 = glue.tile([128, mm, 3], f32)
                nc.scalar.activation(
                    out=e3, in_=pbs, func=ACT_F.Exp, scale=1.0 / 16.0
                )
                ssum = glue.tile([128, mm], f32)
                nc.vector.reduce_sum(ssum, e3, axis=mybir.AxisListType.X)
                rr = glue.tile([128, mm], f32)
                nc.vector.reciprocal(rr, ssum)
                aw3 = glue.tile([128, mm, 3], f32)
                for c in range(3):
                    nc.vector.tensor_mul(aw3[:, :, c], e3[:, :, c], rr)
                nc.sync.dma_start(
                    out=aw_d.ap().rearrange("(a q) c -> a (q c)", a=128),
                    in_=aw3.rearrange("p q c -> p (q c)"),
                )
            rep_ctx.close()

    nc.compile()
    return nc


def make_in_maps(Z, W_rff, A, n_cores: int = N_CORES):
    """Host-side prep: fp16-split + transpose Z, fold constants."""
    Z = np.ascontiguousarray(np.asarray(Z, dtype=np.float32))
    W_rff = np.ascontiguousarray(np.asarray(W_rff, dtype=np.float32))
    A = np.ascontiguousarray(np.asarray(A, dtype=np.float32))
    n_total = Z.shape[0]
    m = n_total // n_cores

    hp = np.float16
    Zh = Z.astype(hp)
    Zl = (Z - Zh.astype(np.float32)).astype(hp)
    z2 = np.ascontiguousarray(np.concatenate([Zh.T, Zl.T], axis=0))
    Wh = W_rff.astype(hp)
    Wl = (W_rff - Wh.astype(np.float32)).astype(hp)
    wx = np.ascontiguousarray(np.concatenate([Wl, Wh], axis=0))

    a_div = (A / 16.0).astype(np.float32)
    a4 = np.ascontiguousarray(
        a_div.reshape(4, 128, 3).transpose(1, 0, 2).reshape(128, 12)
    )
    a_mean = A.mean(axis=1)
    denom = 16.0 * float(n_total) * (0.1 * np.sqrt(RFF_DIM))
    c01 = np.tile(
        np.array([[a_mean[0] / denom, a_mean[1] / denom]], dtype=np.float32),
        (4, 1),
    )
    in_maps = []
    for core in range(n_cores):
        in_maps.append(
            {
                "z2": np.ascontiguousarray(z2[:, core * m : (core + 1) * m]),
                "wh": np.ascontiguousarray(Wh),
                "wx": wx,
                "a4": a4,
                "c01": c01,
            }
        )
    return in_maps, m


_CACHE = {}


def _get_runner(
    m: int, n_cores: int = N_CORES, reduce_mode: str = "k8", reps: int = 1
):
    """Compile once; return (run_fn, put_fn, jitted). Mirrors the multi-core
    branch of bass2jax.run_bass_via_pjrt but caches the jitted callable."""
    key = (m, n_cores, reduce_mode, reps)
    if key in _CACHE:
        return _CACHE[key]

    import jax
    from jax.sharding import Mesh, PartitionSpec, NamedSharding
    from jax.experimental.shard_map import shard_map
    import concourse.mybir as mybir_
    from concourse import bass2jax

    nc = build_module(m, n_cores, reduce_mode, reps=reps, use_cc=(reps == 1))
    bass2jax.install_neuronx_cc_hook()

    partition_name = (
        nc.partition_id_tensor.name if nc.partition_id_tensor else None
    )
    in_names, out_names, out_avals, zero_outs = [], [], [], []
    for alloc in nc.m.functions[0].allocations:
        if not isinstance(alloc, mybir_.MemoryLocationSet):
            continue
        name = alloc.memorylocations[0].name
        if alloc.kind == "ExternalInput":
            if name != partition_name:
                in_names.append(name)
        elif alloc.kind == "ExternalOutput":
            shape = tuple(alloc.tensor_shape)
            dtype = mybir_.dt.np(alloc.dtype)
            out_names.append(name)
            out_avals.append(jax.core.ShapedArray(shape, dtype))
            zero_outs.append(np.zeros(shape, dtype))
    n_params = len(in_names)
    all_in_names = list(in_names) + list(out_names)
    if partition_name is not None:
        all_in_names.append(partition_name)

    def _body(*args):
        operands = list(args)
        if partition_name is not None:
            operands.append(bass2jax.partition_id_tensor())
        outs = bass2jax._bass_exec_p.bind(
            *operands,
            out_avals=tuple(out_avals),
            in_names=tuple(all_in_names),
            out_names=tuple(out_names),
            lowering_input_output_aliases=(),
            sim_require_finite=True,
            sim_require_nnan=True,
            nc=nc,
        )
        return tuple(outs)

    devices = jax.devices()[:n_cores]
    mesh = Mesh(np.asarray(devices), ("core",))
    in_specs = (PartitionSpec("core"),) * (n_params + len(out_names))
    out_specs = (PartitionSpec("core"),) * len(out_names)
    sharded = jax.jit(
        shard_map(_body, mesh=mesh, in_specs=in_specs, out_specs=out_specs,
                  check_rep=False),
        keep_unused=True,
    )
    sharding = NamedSharding(mesh, PartitionSpec("core"))

    def put_fn(in_maps):
        import jax as _jax

        concat_in = [
            np.concatenate([in_maps[c][nm] for c in range(n_cores)], axis=0)
            for nm in in_names
        ]
        concat_zeros = [
            np.zeros((n_cores * z.shape[0], *z.shape[1:]), z.dtype)
            for z in zero_outs
        ]
        return [
            _jax.device_put(a, sharding) for a in concat_in + concat_zeros
        ]

    def run_fn(device_args):
        out_arrs = sharded(*device_args)
        out_arrs = [np.asarray(o) for o in out_arrs]
        return [
            {
                nm: out_arrs[i].reshape(n_cores, *out_avals[i].shape)[c]
                for i, nm in enumerate(out_names)
            }
            for c in range(n_cores)
        ]

    _CACHE[key] = (run_fn, put_fn, sharded)
    return _CACHE[key]


def kernel(Z, W_rff, A):
    import time

    Z = np.asarray(Z, dtype=np.float32)
    A_np = np.asarray(A, dtype=np.float32)
    in_maps, m = make_in_maps(Z, W_rff, A_np, N_CORES)
    run_fn, put_fn, _ = _get_runner(m, N_CORES)
    dev_args = put_fn(in_maps)
    try:
        results = run_fn(dev_args)
    except Exception:
        # the axon-tunneled devices occasionally come up wedged from a
        # previous session; one retry after a pause usually recovers
        time.sleep(10.0)
        results = run_fn(dev_args)
    preds = np.concatenate([r["preds"] for r in results], axis=0)
    aw = np.concatenate([r["aw"] for r in results], axis=0)
    alpha = results[0]["alpha"].reshape(-1)
    reg_loss = np.float32(
        0.01 * np.linalg.svd(A_np, compute_uv=False).astype(np.float32).sum()
    )
    return preds, aw, reg_loss, alpha
